# revision 1
# baseline (speedup 1.0000x reference)
"""Trainium2 Bass kernel: single-channel 2D conv (valid), X[8192,8192] * w[5,5] + bias.

Strategy: row-shard X across 8 NeuronCores with a (kh-1)-row halo (host-side
overlapping slices; weight/bias replicated). On each core, the conv is computed
as 5 PSUM-accumulated TensorE matmuls per output tile: for each kernel column
dj, a banded stationary matrix A_dj[k, m] = w[k-m, dj] (0 <= k-m < 5) contracts
over 128 input rows to produce 124 output rows of the column-direction conv,
while the moving operand is the input tile shifted by dj columns. Accumulating
the 5 dj-shifts in PSUM yields the full 5x5 conv. fp32r (hardware rounds
operands to 11 mantissa bits, fp32 accumulate) runs the PE at 1 cycle/row.
"""

import numpy as np

import concourse.bass as bass
import concourse.mybir as mybir
from concourse import bacc
from concourse import bass_utils
from concourse.tile import TileContext

H = 8192
W = 8192
KH = 5
KW = 5
OH = H - KH + 1  # 8188
OW = W - KW + 1  # 8188

NCORES = 8
ROWS_OUT = 1024  # output rows per core (8*1024 = 8192 >= 8188; tail cropped)
ROWS_IN = ROWS_OUT + KH - 1  # 1028

BAND_OUT = 124  # output rows per matmul band (K=128 partitions -> M=124)
SUB_W = 512  # matmul moving free dim (one PSUM bank of fp32)

# 8 full bands of 124 output rows + a 32-row tail band (fp32r handles M=32)
_BANDS = [(124 * i, 124) for i in range(8)] + [(992, 32)]
# 16 uniform column subtiles; the last one overlaps
_SUB_STARTS = [512 * i for i in range(15)] + [OW - SUB_W]

_PROGRAM_CACHE = {}

# Populated by the most recent kernel() call when tracing is enabled via
# TRACE=1 (module attr) — used by test.py for HW exec time reporting.
TRACE = False
LAST_RUN = {}


def _build_program(bias_val: float):
    f32 = mybir.dt.float32
    f32r = mybir.dt.float32r

    nc = bacc.Bacc("TRN2", target_bir_lowering=False, debug=False, num_devices=NCORES)

    Xs = nc.dram_tensor("Xs", [ROWS_IN, W], f32r, kind="ExternalInput")
    Aw = nc.dram_tensor("Aw", [128, KW * BAND_OUT], f32r, kind="ExternalInput")
    # Output rows padded to 8192 cols so every store row is a 32KiB-aligned
    # full-line HBM write; host crops to 8188.
    Y = nc.dram_tensor("Y", [ROWS_OUT, W], f32, kind="ExternalOutput")

    with TileContext(nc) as tc:
        with (
            tc.tile_pool(name="const", bufs=1) as cpool,
            tc.tile_pool(name="inp", bufs=3) as in_pool,
            tc.tile_pool(name="outp", bufs=2) as out_pool,
            tc.tile_pool(name="psum", bufs=8, space="PSUM") as psum_pool,
        ):
            A_t = cpool.tile([128, KW * BAND_OUT], f32r)
            nc.sync.dma_start(A_t[:], Aw.ap())

            # DRAM->SBUF loads spread across all 16 SDMA engines; SBUF->DRAM
            # stores concentrate on few engines per instruction, so issue
            # stores as many small instructions alternating across the two
            # HWDGE rings to engage more engines.
            # Topology: loads on the gpsimd SWDGE queue (32KiB descriptors,
            # spreads over all 16 SDMA engines, never blocked behind
            # compute-dependent stores). Stores mostly on the two HWDGE rings
            # (fast but pinned to SDMA engines 64-71); ~20% of store rows
            # offloaded to SWDGE (deferred one band so they don't block the
            # load issue stream) to relieve the hot engines.
            qs = [nc.sync, nc.scalar]
            pending = []
            for bi, (r0, rows_out) in enumerate(_BANDS):
                rows_in = rows_out + KH - 1
                in_t = in_pool.tile([rows_in, W], f32r)
                nc.gpsimd.dma_start(in_t[:], Xs.ap()[r0 : r0 + rows_in, :])
                if pending:
                    r0s, sw_rows, t = pending.pop(0)
                    nc.gpsimd.dma_start(Y.ap()[r0s : r0s + sw_rows, :], t[0:sw_rows, :])
                out_t = out_pool.tile([rows_out, W], f32)
                for c0 in _SUB_STARTS:
                    ps = psum_pool.tile([rows_out, SUB_W], f32)
                    for dj in range(KW):
                        nc.tensor.matmul(
                            ps[:],
                            A_t[0:rows_in, dj * BAND_OUT : dj * BAND_OUT + rows_out],
                            in_t[:, c0 + dj : c0 + dj + SUB_W],
                            start=(dj == 0),
                            stop=(dj == KW - 1),
                        )
                    dst = out_t[:, c0 : c0 + SUB_W]
                    if bias_val == 0.0:
                        nc.vector.tensor_copy(dst, ps[:])
                    else:
                        nc.scalar.activation(
                            dst,
                            ps[:],
                            mybir.ActivationFunctionType.Copy,
                            bias=bias_val,
                        )
                sw_rows = 24 if rows_out == BAND_OUT else 8
                pending.append((r0, sw_rows, out_t))
                n_chunks = 12 if rows_out == BAND_OUT else 4
                lo0 = sw_rows
                bounds = [
                    lo0 + (rows_out - lo0) * i // n_chunks for i in range(n_chunks + 1)
                ]
                for ci in range(n_chunks):
                    lo, hi = bounds[ci], bounds[ci + 1]
                    qs[ci % 2].dma_start(
                        Y.ap()[r0 + lo : r0 + hi, :], out_t[lo:hi, :]
                    )
            while pending:
                r0s, sw_rows, t = pending.pop(0)
                nc.gpsimd.dma_start(Y.ap()[r0s : r0s + sw_rows, :], t[0:sw_rows, :])

    nc.compile()
    return nc


def kernel(X, weight, bias):
    X = np.ascontiguousarray(np.asarray(X, dtype=np.float32))
    weight = np.asarray(weight, dtype=np.float32)
    bias = np.asarray(bias, dtype=np.float32)
    assert X.shape == (H, W) and weight.shape == (KH, KW)

    bias_val = float(bias.reshape(-1)[0])
    key = bias_val
    nc = _PROGRAM_CACHE.get(key)
    if nc is None:
        nc = _build_program(bias_val)
        _PROGRAM_CACHE[key] = nc

    # Banded stationary matrices: A[k, dj*124 + m] = w[k-m, dj] for 0<=k-m<5
    A = np.zeros((128, KW * BAND_OUT), dtype=np.float32)
    m = np.arange(BAND_OUT)
    for dj in range(KW):
        for di in range(KH):
            A[m + di, dj * BAND_OUT + m] = weight[di, dj]

    # Row-shard with halo; pad the bottom so every core gets ROWS_IN rows.
    Xp = np.zeros((NCORES * ROWS_OUT + KH - 1, W), dtype=np.float32)
    Xp[:H] = X
    in_maps = [
        {"Xs": Xp[c * ROWS_OUT : c * ROWS_OUT + ROWS_IN], "Aw": A}
        for c in range(NCORES)
    ]

    res = bass_utils.run_bass_kernel_spmd(
        nc, in_maps, core_ids=list(range(NCORES)), trace=TRACE
    )
    LAST_RUN.clear()
    LAST_RUN.update(
        exec_time_ns=res.exec_time_ns,
        instructions_and_trace=res.instructions_and_trace,
        profile_json=res.profile_json,
    )

    out = np.concatenate([res.results[c]["Y"] for c in range(NCORES)], axis=0)
    return np.ascontiguousarray(out[:OH, :OW])

